# revision 59
# baseline (speedup 1.0000x reference)
"""Multi-head attention (B=4, N=2048, D=768, H=12) on 8 trn2 NeuronCores.

Sharding: core c -> (batch b = c//2, head-half g = c%2).  Each core computes
the qkv projection for its 6 heads, attention, and a partial output
projection (over its 384 feature columns).  The host sums the two fp16
partials per batch and adds the proj bias.  No collectives.

All heavy matmuls run in bfloat16 (1 cyc/col moving operand vs 2 for
fp32r) with fp32 PSUM accumulation; end-to-end rel err ~1e-2 vs the 2e-2
gate.  ~309us vs the 480us fp32r baseline measured back-to-back.

Device design (per core):
 - x is transposed on host to xT [768, 2048] bf16 so the contraction dim is
   on SBUF partitions for both the Q/K (xT as rhs) and V (xT as lhsT)
   matmuls.  Input DMAs round-robin across three DGE queues (sync, ACT,
   gpsimd) with the first-needed pieces (wq k-cols, xT chunk 0) first.
 - A ~4.3us burst of dummy ones-matmuls at t=0 flips the PE HAM clock gate
   to 8/8 before real work arrives (otherwise the qkv phase runs at 1.2GHz).
 - Q^T/K^T are produced as per-pair [128, 2048] bf16 tiles (head-dim on
   partitions; rows 0-63 = head 2p, 64-127 = head 2p+1), enabling row-tiled
   (K=64 x2) concurrent S^T matmuls.
 - S^T = K Q^T per (pair, k-tile, q-chunk); exp runs on ACT directly from
   PSUM with scale=1/8 folded in (no max subtraction: |scores*scale| < ~7),
   writing bf16.  ACT is near-saturated (~211us of exp): the attention
   phase is scheduled around keeping the 192-deep exp stream unbroken.
 - AV is column-tiled: v_h0 at tile_position (0,0) -> psum partitions 0-63
   and v_h1 at (0,64) -> partitions 64-127 stream their a_t chunks
   concurrently (4-XBUS col tiling), halving AV wall time vs M=65 one-head
   matmuls.  Softmax denominators come from a bf16 pairwise-sum tree over
   the a_t tiles on DVE plus one M=1 ones-matmul per head into a shared
   psum bank (partition 0 / 64).
 - Normalization: reciprocal_approx_fast reads the denominators straight
   from PSUM at base partition 0 (the custom-DVE uop misbehaves at nonzero
   base partitions), bf16 broadcast via K=1 matmul with a ones-row lhsT,
   then two partition-aligned DVE multiplies build the proj lhsT layout
   directly (even head -> rows 0-63, odd head -> rows 64-127).
 - Per-pair normalization is deferred into the next pair's kt loop so the
   reciprocal chain never gates the PE queue; the final q-chunk's norm+proj
   run out of the freed ps_s pool (bufs=2) to unserialize the tail, with
   the last output DMAs split across all three queues.
"""

import numpy as np
import ml_dtypes

import concourse.bacc as bacc
import concourse.bass as bass  # noqa: F401
import concourse.mybir as mybir
import concourse.tile as tile
from concourse.bass_utils import run_bass_kernel_spmd

P = 128
NQ = 2048          # sequence length
CD = 768           # model dim
NHC = 6            # heads per core
DH = 64            # head dim
SCALE = DH ** -0.5
CT = CD // P       # 6 c-tiles
KT = NQ // P       # 16 k-tiles
QC = 512           # q chunk
NQC = NQ // QC     # 4
PAIRS = NHC // 2   # 3

F32 = mybir.dt.float32
F16 = mybir.dt.float16     # output partials: halves out DMA, ~5e-4 rounding
BF16 = mybir.dt.bfloat16   # 1 cyc/col moving operand


def build_nc(n_reps=1, debug=False):
    nc = bacc.Bacc("TRN2", debug=False, num_devices=8)

    xT_d = nc.dram_tensor("xT", [CD, NQ], BF16, kind="ExternalInput")
    wqkvT_d = nc.dram_tensor("wqkvT", [CD, 3 * 384], BF16, kind="ExternalInput")
    bqk_d = nc.dram_tensor("b_qk", [P, 6], F32, kind="ExternalInput")
    bv_d = nc.dram_tensor("b_v", [1, 384], BF16, kind="ExternalInput")
    wpT_d = nc.dram_tensor("wpT", [384, CD], BF16, kind="ExternalInput")
    ones_d = nc.dram_tensor("ones", [P, P], BF16, kind="ExternalInput")
    out_d = nc.dram_tensor("out", [NQ, CD], F16, kind="ExternalOutput")
    if debug:
        qk_dbg = nc.dram_tensor("qk_dbg", [2, PAIRS, P, NQ], BF16,
                                kind="ExternalOutput")
        v_dbg = nc.dram_tensor("v_dbg", [P, KT, PAIRS, 193], BF16,
                               kind="ExternalOutput")
        at_dbg = nc.dram_tensor("at_dbg", [NQC, P, PAIRS, QC], BF16,
                                kind="ExternalOutput")
        avc_dbg = nc.dram_tensor("avc_dbg", [NQC, PAIRS, P, QC], F32,
                                 kind="ExternalOutput")
        rc_dbg = nc.dram_tensor("rc_dbg", [NQC, PAIRS, 2, QC], F32,
                                kind="ExternalOutput")

    with tile.TileContext(nc) as tc:
        with (
            tc.tile_pool(name="consts", bufs=1) as consts,
            tc.tile_pool(name="big", bufs=1) as big,
            tc.tile_pool(name="attn", bufs=2) as attn_pool,
            tc.tile_pool(name="aT", bufs=6) as aT_pool,
            tc.tile_pool(name="norm", bufs=1) as norm_pool,
            tc.tile_pool(name="outst", bufs=2) as outst_pool,
            tc.tile_pool(name="ps_s", bufs=2, space="PSUM") as ps_s,
            tc.tile_pool(name="ps_av", bufs=1, space="PSUM") as ps_av,
            tc.tile_pool(name="ps_d", bufs=1, space="PSUM") as ps_d,
            tc.tile_pool(name="ps_proj", bufs=1, space="PSUM") as ps_proj,
            tc.tile_pool(name="dsum", bufs=2) as dsum_pool,
        ):
            # ---- constants (per-c-tile tiles so compute starts ASAP) ----
            xT_sb = [consts.tile([P, NQ], BF16, tag=f"xT{ct}", name=f"xT{ct}")
                     for ct in range(CT)]
            wq_sb = [consts.tile([P, 3 * 384], BF16, tag=f"wqkvT{ct}",
                                 name=f"wqkvT{ct}") for ct in range(CT)]
            # round-robin input DMAs across three DGE queues (sync + ACT +
            # gpsimd; ACT and gpsimd are idle at startup) so the ~5.4MB
            # input load approaches the 358 GB/s HBM limit
            _dma_rr = [0]
            _dma_engs = (nc.sync, nc.scalar, nc.gpsimd)

            def dma_in(dst, src):
                eng = _dma_engs[_dma_rr[0] % 3]
                _dma_rr[0] += 1
                eng.dma_start(dst, src)

            def dma_w_piece(piece):
                for ct in range(CT):
                    dma_in(
                        wq_sb[ct][:, piece * 384:(piece + 1) * 384],
                        wqkvT_d[ct * P:(ct + 1) * P, piece * 384:(piece + 1) * 384])

            def dma_x_half(h):
                # [128, 1024] transfers: 2KB per partition line, the knee of
                # DMA line-size efficiency
                for ct in range(CT):
                    dma_in(
                        xT_sb[ct][:, h * 2 * QC:(h + 1) * 2 * QC],
                        xT_d[ct * P:(ct + 1) * P, h * 2 * QC:(h + 1) * 2 * QC])

            # tiny constants first: the ones tile feeds the HAM warmup burst
            ones_sb = consts.tile([P, P], BF16, tag="ones")
            nc.sync.dma_start(ones_sb[:, :], ones_d[:, :])
            bqk_sb = consts.tile([P, 6], F32, tag="bqk")
            nc.sync.dma_start(bqk_sb[:, :], bqk_d[:, :])
            bv_sb = consts.tile([1, 384], BF16, tag="bv")
            nc.sync.dma_start(bv_sb[:, :], bv_d[:, :])
            # -ln(64) exp bias (softmax-invariant; keeps sums in range)
            expb_sb = consts.tile([P, 1], F32, tag="expb")
            nc.vector.memset(expb_sb[:, :], -4.1588830833596715)

            # HAM warmup: ~4.3us of dummy matmuls at the cold 1.2 GHz clock
            # flips the PE clock gate to 8/8 before the real qkv work arrives
            # (input DMA takes ~15us; without this the whole qkv phase runs
            # at half clock).  The operand is a memset tile so the burst
            # starts immediately instead of waiting on the ones DMA.
            WARMUP = 40
            if WARMUP:
                wusrc = consts.tile([P, P], BF16, tag="wusrc")
                nc.vector.memset(wusrc[:, :], 1.0)
                wu = ps_s.tile([P, 2, QC], F32, tag="s")
                for _ in range(WARMUP):
                    nc.tensor.matmul(wu[:, 0, 0:P], lhsT=wusrc[:, :],
                                     rhs=wusrc[:, :], start=True, stop=True)

            # interleaved so the first attention batch's dependencies (wq
            # k-piece, xT first half, then the q-piece) land first
            dma_w_piece(1)        # k cols
            dma_x_half(0)
            dma_w_piece(0)        # q cols
            dma_x_half(1)
            dma_w_piece(2)        # v cols
            wp_sb = []
            for t3 in range(3):
                w = consts.tile([P, CD], BF16, tag=f"wpT{t3}")
                dma_in(w[:, :], wpT_d[t3 * P:(t3 + 1) * P, :])
                wp_sb.append(w)

            for _rep in range(n_reps):
                # ---- persistent activations ----
                # per-pair Q^T/K^T [128, 2048]: rows 0-63 head 2p, 64-127 head 2p+1
                q_sb = [big.tile([P, NQ], BF16, tag=f"q{p}", name=f"q{p}") for p in range(PAIRS)]
                k_sb = [big.tile([P, NQ], BF16, tag=f"k{p}", name=f"k{p}") for p in range(PAIRS)]
                # v[part=k-position, k-tile, pair, 193]: cols 0:64 = v_h0,
                # cols 129:193 = v_h1.  The AV matmul is column-tiled: v_h0
                # at tile_position (0,0) -> psum partitions 0-63, v_h1 at
                # (0,64) -> partitions 64-127, concurrently in one psum bank.
                # Softmax denominators no longer ride an appended ones column
                # -- they come from a bf16 pairwise-sum tree over the a_t
                # tiles on DVE plus one M=1 ones-matmul per head.
                VW = 193
                v_sb = big.tile([P, KT, PAIRS, VW], BF16, tag="v")

                def qk_unit(kind, t, qc):
                    # Q^T (kind 0) / K^T (kind 1) pair-tile t, one 512-chunk
                    dest = (q_sb if kind == 0 else k_sb)[t]
                    col0 = kind * 384 + t * P
                    ps = ps_s.tile([P, 2, QC], F32, tag="s")
                    for ct in range(CT):
                        nc.tensor.matmul(
                            ps[:, 0, :],
                            lhsT=wq_sb[ct][:, col0:col0 + P],
                            rhs=xT_sb[ct][:, qc * QC:(qc + 1) * QC],
                            start=(ct == 0),
                            stop=(ct == CT - 1),
                        )
                    nc.vector.tensor_scalar_add(
                        out=dest[:, qc * QC:(qc + 1) * QC],
                        in0=ps[:, 0, :],
                        scalar1=bqk_sb[:, kind * 3 + t:kind * 3 + t + 1],
                    )

                def v_tile(nt):
                    ps = ps_s.tile([P, 2, QC], F32, tag="s")
                    for ct in range(CT):
                        nc.tensor.matmul(
                            ps[:, 0, 0:384],
                            lhsT=xT_sb[ct][:, nt * P:(nt + 1) * P],
                            rhs=wq_sb[ct][:, 768:1152],
                            start=(ct == 0),
                            stop=False,
                        )
                    # bias via K=1 ones-row matmul
                    nc.tensor.matmul(
                        ps[:, 0, 0:384],
                        lhsT=ones_sb[0:1, :],
                        rhs=bv_sb[0:1, :],
                        start=False,
                        stop=True,
                    )
                    vh = ps[:, 0, 0:384].rearrange("p (a b d) -> p a b d",
                                                   a=PAIRS, b=2)
                    nc.vector.tensor_copy(
                        out=v_sb[:, nt, :, 0:DH],
                        in_=vh[:, :, 0, :],
                    )
                    nc.vector.tensor_copy(
                        out=v_sb[:, nt, :, 2 * DH + 1:VW],
                        in_=vh[:, :, 1, :],
                    )

                at_chunks = {}

                def attn_pair(qc, pr, emit_v=False, pending=None, last=False):
                    qsl = slice(qc * QC, (qc + 1) * QC)
                    if pr == 0:
                        at_chunks[qc] = attn_pool.tile([P, PAIRS, QC], BF16,
                                                       tag="attnT", name="at_chunk")
                    at_chunk = at_chunks[qc]
                    av = ps_av.tile([P, QC], F32, tag="av")
                    LOOKAHEAD = 2   # S^T/exp run ahead of AV so the PE queue
                    a_ts = {}       # has work while the av slot drains
                    # denominator pairwise-sum tree buffers (bf16)
                    t8 = dsum_pool.tile([P, 8, 2, QC], BF16, tag="t8")
                    t4 = dsum_pool.tile([P, 4, 2, QC], BF16, tag="t4")
                    t2 = dsum_pool.tile([P, 2, 2, QC], BF16, tag="t2")
                    t1 = dsum_pool.tile([P, 2, QC], BF16, tag="t1")

                    def st_exp(kt):
                        if emit_v:
                            v_tile(kt)
                        sp = ps_s.tile([P, 2, QC], F32, tag="s")
                        for h2 in range(2):
                            nc.tensor.matmul(
                                sp[:, h2, :],
                                lhsT=k_sb[pr][h2 * DH:(h2 + 1) * DH,
                                              kt * P:(kt + 1) * P],
                                rhs=q_sb[pr][h2 * DH:(h2 + 1) * DH, qsl],
                                start=True,
                                stop=True,
                                tile_position=(h2 * DH, 0),
                            )
                        a_t = aT_pool.tile([P, 2, QC], BF16, tag="aT")
                        # -ln(64) bias keeps sums well in range
                        # (softmax is invariant to this common scaling)
                        nc.scalar.activation(
                            out=a_t[:, :, :],
                            in_=sp[:, :, :],
                            func=mybir.ActivationFunctionType.Exp,
                            bias=expb_sb[:, 0:1],
                            scale=float(SCALE),
                        )
                        a_ts[kt] = a_t
                        if kt % 2 == 1:
                            # (GPSIMD offload of these adds measured +88us --
                            # its software pipelining is far below the DVE)
                            nc.vector.tensor_add(
                                out=t8[:, kt // 2, :, :],
                                in0=a_ts[kt - 1][:, :, :],
                                in1=a_t[:, :, :],
                            )

                    def av_mm(kt):
                        a_t = a_ts.pop(kt)
                        # column-tiled pair: both heads stream concurrently
                        nc.tensor.matmul(
                            av[0:DH, :],
                            lhsT=v_sb[:, kt, pr, 0:DH],
                            rhs=a_t[:, 0, :],
                            start=(kt == 0),
                            stop=(kt == KT - 1),
                            tile_position=(0, 0),
                        )
                        nc.tensor.matmul(
                            av[DH:P, :],
                            lhsT=v_sb[:, kt, pr, 2 * DH + 1:VW],
                            rhs=a_t[:, 1, :],
                            start=(kt == 0),
                            stop=(kt == KT - 1),
                            tile_position=(0, DH),
                        )

                    def tree_l2(j):
                        nc.vector.tensor_add(out=t4[:, j, :, :],
                                             in0=t8[:, 2 * j, :, :],
                                             in1=t8[:, 2 * j + 1, :, :])

                    for kt in range(KT):
                        st_exp(kt)
                        # level-2 tree adds spread into the loop as their
                        # inputs complete, so the pair-end serial DVE chain
                        # is 4 ops instead of 7
                        if kt in (5, 9, 13):
                            tree_l2((kt - 5) // 4)
                        if kt == 5 and pending is not None:
                            pending()   # previous pair's deferred normalization
                        if kt >= LOOKAHEAD:
                            av_mm(kt - LOOKAHEAD)
                    for kt in range(KT - LOOKAHEAD, KT):
                        av_mm(kt)
                    # finish the denominator tree and reduce over partitions
                    # with one M=1 ones-matmul per head
                    tree_l2(3)
                    for j in range(2):
                        nc.vector.tensor_add(out=t2[:, j, :, :],
                                             in0=t4[:, 2 * j, :, :],
                                             in1=t4[:, 2 * j + 1, :, :])
                    nc.vector.tensor_add(out=t1[:, :, :], in0=t2[:, 0, :, :],
                                         in1=t2[:, 1, :, :])
                    # one psum bank: h0's denominator at partition 0, h1's at
                    # partition 64 (column tile_position), so the downstream
                    # ops read at legal base partitions
                    d_ps = ps_d.tile([P, QC], F32, tag="d")
                    for h2 in range(2):
                        nc.tensor.matmul(
                            d_ps[h2 * DH:h2 * DH + 1, :],
                            lhsT=ones_sb[:, 0:1],
                            rhs=t1[:, h2, :],
                            start=True,
                            stop=True,
                            tile_position=(0, h2 * DH),
                        )
                    # evacuate av psum early (frees the slot for the next
                    # pair) on DVE -- ACT is the bottleneck engine; the
                    # normalization itself is deferred into the next pair's
                    # kt loop so the reciprocal chain never gates the PE queue
                    avc = norm_pool.tile([P, QC], F32, tag="avc")
                    nc.vector.tensor_copy(out=avc[:, :], in_=av[:, :])
                    if debug:
                        nc.sync.dma_start(avc_dbg[qc, pr, :, :], avc[:, :])

                    def norm():
                        # recip rows 0 and 64 (lanes 1-63 compute garbage on
                        # never-written psum, harmless and unread)
                        rc2 = norm_pool.tile([DH + 1, QC], F32, tag="rc2")
                        rc2b = norm_pool.tile([DH + 1, QC], BF16, tag="rc2b")
                        with nc.allow_low_precision(reason="softmax denom recip"):
                            nc.vector.reciprocal_approx_fast(
                                out=rc2[:, :], in_=d_ps[0:DH + 1, :])
                            nc.vector.tensor_copy(out=rc2b[0:1, :],
                                                  in_=rc2[0:1, :])
                            nc.vector.tensor_copy(out=rc2b[DH:DH + 1, :],
                                                  in_=rc2[DH:DH + 1, :])
                        if debug:
                            nc.sync.dma_start(rc_dbg[qc, pr, 0, :], rc2[0, :])
                            nc.sync.dma_start(rc_dbg[qc, pr, 1, :],
                                              rc2[DH, :])
                        # the final pair's norm runs after all attention --
                        # ps_s is free then, and its 2 bufs unserialize the
                        # tail norm+proj chain
                        bc_pool, bc_tag = (ps_s, "s") if last else (ps_proj, "proj")
                        for h2 in range(2):
                            bc_ps = bc_pool.tile([P, 2, QC], F32, tag=bc_tag)
                            nc.tensor.matmul(
                                bc_ps[:, h2, :],
                                lhsT=ones_sb[h2 * DH:h2 * DH + 1, :],
                                rhs=rc2b[h2 * DH:h2 * DH + 1, :],
                                start=True,
                                stop=True,
                            )
                            if h2 == 0:
                                nc.vector.tensor_mul(
                                    out=at_chunk[0:DH, pr, :],
                                    in0=avc[0:DH, :],
                                    in1=bc_ps[0:DH, 0, :],
                                )
                            else:
                                nc.vector.tensor_mul(
                                    out=at_chunk[DH:P, pr, :],
                                    in0=avc[DH:P, :],
                                    in1=bc_ps[DH:P, 1, :],
                                )
                    return norm

                def proj_chunk(qc):
                    at_chunk = at_chunks[qc]
                    if debug:
                        nc.sync.dma_start(at_dbg[qc, :, :, :], at_chunk[:, :, :])
                    last = qc == NQC - 1
                    pool, tag = (ps_s, "s") if last else (ps_proj, "proj")
                    for sub in range(QC // P):
                        pp = pool.tile([P, 2, QC], F32, tag=tag)
                        for t3 in range(PAIRS):
                            for (bank, o0, ow) in ((0, 0, 512), (1, 512, 256)):
                                nc.tensor.matmul(
                                    pp[:, bank, 0:ow],
                                    lhsT=at_chunk[:, t3, sub * P:(sub + 1) * P],
                                    rhs=wp_sb[t3][:, o0:o0 + ow],
                                    start=(t3 == 0),
                                    stop=(t3 == PAIRS - 1),
                                )
                        ost = outst_pool.tile([P, CD], F16, tag="ost")
                        if last:
                            # ACT is idle in the tail: split the psum
                            # evacuation across both engines
                            nc.vector.tensor_copy(out=ost[:, 0:512], in_=pp[:, 0, :])
                            nc.scalar.copy(out=ost[:, 512:CD], in_=pp[:, 1, 0:256])
                        else:
                            nc.vector.tensor_copy(out=ost[:, 0:512], in_=pp[:, 0, :])
                            nc.vector.tensor_copy(out=ost[:, 512:CD], in_=pp[:, 1, 0:256])
                        n0 = qc * QC + sub * P
                        if last:
                            # drain the final output across three queues,
                            # split by rows so the 1536B DMA lines survive
                            nc.sync.dma_start(out_d[n0:n0 + 44, :],
                                              ost[0:44, :])
                            nc.scalar.dma_start(out_d[n0 + 44:n0 + 88, :],
                                                ost[44:88, :])
                            nc.gpsimd.dma_start(out_d[n0 + 88:n0 + P, :],
                                                ost[88:P, :])
                        else:
                            nc.sync.dma_start(out_d[n0:n0 + P, :], ost[:, :])

                def qk_tile(kind, t):
                    for qc in range(NQC):
                        qk_unit(kind, t, qc)

                # software-pipelined emission: K pair0 + Q pair0 chunk0 are
                # all that chunk-0/pair-0 attention needs, so emit them first
                # (with V production interleaved per k-tile) to start ACT as
                # early as the xT DMA allows.
                # (interleaved warm-fill matmuls between these units were
                # tried and measured slightly WORSE -- the extra psum-slot
                # rotations cost more than the clock-gate win)
                qk_tile(1, 0)
                qk_unit(0, 0, 0)
                pending = attn_pair(0, 0, emit_v=True)
                for qc in range(1, NQC):
                    qk_unit(0, 0, qc)
                for pr in range(1, PAIRS):
                    qk_tile(0, pr)
                    qk_tile(1, pr)
                    pending = attn_pair(0, pr, pending=pending)
                pending()
                pending = None
                proj_chunk(0)
                for qc in range(1, NQC):
                    for pr in range(PAIRS):
                        last = qc == NQC - 1 and pr == PAIRS - 1
                        pending = attn_pair(qc, pr, pending=pending, last=last)
                    pending()
                    pending = None
                    proj_chunk(qc)
                if debug:
                    for p in range(PAIRS):
                        nc.sync.dma_start(qk_dbg[0, p, :, :], q_sb[p][:, :])
                        nc.sync.dma_start(qk_dbg[1, p, :, :], k_sb[p][:, :])
                    nc.sync.dma_start(v_dbg[:, :, :, :], v_sb[:, :, :, :])

    nc.finalize()
    return nc


_NC = None


def _get_nc():
    global _NC
    if _NC is None:
        _NC = build_nc()
    return _NC


def _make_in_maps(inputs):
    x = np.asarray(inputs["x"], dtype=np.float32)
    w_qkv = np.asarray(inputs["w_qkv"], dtype=np.float32)
    b_qkv = np.asarray(inputs["b_qkv"], dtype=np.float32)
    w_proj = np.asarray(inputs["w_proj"], dtype=np.float32)
    BF = ml_dtypes.bfloat16

    in_maps = []
    for c in range(8):
        b, g = c // 2, c % 2
        sl = slice(384 * g, 384 * g + 384)
        xT = np.ascontiguousarray(x[b].T)                       # [768, 2048]
        wq = w_qkv[0:768][sl]                                    # [384, 768]
        wk = w_qkv[768:1536][sl]
        wv = w_qkv[1536:2304][sl]
        wqkvT = np.ascontiguousarray(np.concatenate([wq, wk, wv], axis=0).T)
        bq = b_qkv[0:768][sl]
        bk = b_qkv[768:1536][sl]
        bv = b_qkv[1536:2304][sl]
        b_qk = np.ascontiguousarray(
            np.concatenate([bq, bk]).reshape(6, P).T)            # [128, 6]
        wpT = np.ascontiguousarray(w_proj[:, sl].T)
        in_maps.append({
            "ones": np.ones((P, P), dtype=BF),
            "xT": xT.astype(BF),
            "wqkvT": wqkvT.astype(BF),
            "b_qk": b_qk,
            "b_v": np.ascontiguousarray(bv.reshape(1, 384)).astype(BF),
            "wpT": wpT.astype(BF),
        })
    return in_maps


def _run(inputs, trace=False):
    nc = _get_nc()
    in_maps = _make_in_maps(inputs)
    res = run_bass_kernel_spmd(nc, in_maps, core_ids=list(range(8)), trace=trace)
    b_proj = np.asarray(inputs["b_proj"], dtype=np.float32)
    out = np.empty((4, NQ, CD), dtype=np.float32)
    for b in range(4):
        out[b] = (res.results[2 * b]["out"].astype(np.float32)
                  + res.results[2 * b + 1]["out"].astype(np.float32) + b_proj)
    return out, res


def kernel(**inputs) -> np.ndarray:
    out, _ = _run(inputs, trace=False)
    return out
